# revision 26
# baseline (speedup 1.0000x reference)
"""Fused transformer block (LN-over-sequence + causal MHA + LN + MLP, residuals)
for Trainium2, distributed over 8 NeuronCores.

v2 design (all-bf16 matmul paths; measured rel-err ~5e-3 vs 2e-2 gate):
  Phase A (attention): tensor-parallel over heads -- core c owns heads
    (2c, 2c+1) = channels [128c, 128c+128). Channel-major layouts throughout
    so the sequence-dim LayerNorms reduce along the free axis.
    LN1 is folded into the QKV weights (bf16); x is uploaded once in bf16.
    Softmax without max-subtraction (logits bounded); denominator from an
    appended ones-column in V (attention-psum row 64); reciprocal via the
    fast DVE approx; causality via partial-width diagonal slices + one
    triangular mask multiply.
  Re-shard: per-batch bf16 AllToAlls (payload = x1 + LN2 stats columns).
    Batch-0's collective is triggered as soon as batch-0's attention ends and
    overlaps batch-1 compute; token ownership is interleaved (core c owns
    tokens [256c, 256c+256) of BOTH batches) so the SPMD program reads both
    gather buffers symmetrically.
  Phase B (MLP): token-parallel 1024->4096->1024 on the core's 512 tokens
    (256 per batch, column-packed). LN2 scale/bias are computed receiver-side
    from shipped (mean, var) so the scalar engine's Exp table never swaps
    during attention. W1/W2 are DMA'd at kernel start on the GpSimd queue so
    they never block the input/payload stream.
"""

import numpy as np

import concourse.bass as bass
import concourse.mybir as mybir
import concourse.tile as tile
from concourse import bacc
from concourse.bass_utils import run_bass_kernel_spmd

B, T, C, H, D = 2, 2048, 1024, 16, 64
NCORES = 8
CT = C // 128  # 8 channel tiles
TS = 512  # token slice width (attention loop)
NTS = T // TS  # 4
NST = T // 128  # 16 s-tiles
TB = T // NCORES  # 256 tokens owned per (core, batch)
HID = 4 * C
NJ = HID // 128  # 32 hidden tiles
EPS = 1e-5
VCORR = T / (T - 1.0)  # unbiased-variance correction (torch.var ddof=1)
SCALE = C**-0.5  # attention logit scale = 1/32

F32 = mybir.dt.float32
F32R = mybir.dt.float32r
BF16 = mybir.dt.bfloat16
AF = mybir.ActivationFunctionType
ALU = mybir.AluOpType

_CACHE: dict = {}


def build():
    nc = bacc.Bacc(None, target_bir_lowering=False, debug=False, num_devices=NCORES)

    def din(name, shape, dt=F32):
        return nc.dram_tensor(name, shape, dt, kind="ExternalInput").ap()

    xb_d = din("xb", [B, CT, 128, T], BF16)  # x channel-major, bf16 (shared)
    xow_d = din("xow", [B, 2, 64, T], BF16)  # core's own channels
    # packs laid out [128, 8*128]: col block ct holds rows 128ct..128ct+128
    wqq_d = din("wqq", [128, C], BF16)
    wkk_d = din("wkk", [128, C], BF16)
    wvv_d = din("wvv", [128, C], BF16)
    g1_d = din("g1c", [128, CT], F32)
    be1_d = din("be1c", [128, CT], F32)
    g2_d = din("g2c", [128, CT], F32)
    be2_d = din("be2c", [128, CT], F32)
    b1_d = din("b1c", [128, NJ], F32)
    b2_d = din("b2c", [128, CT], F32)
    tm_d = din("trimask", [128, 128], BF16)  # tri: 1 if t_local >= s_local
    id_d = din("identr", [128, 128], BF16)
    onr_d = din("onesrow", [1, 64], BF16)
    onc_d = din("onescol", [128, 1], BF16)
    w1_d = din("w1t", [CT, 4, 128, 1024], BF16)  # W1 [c-tile, jb, p, 8x128]
    w2_d = din("w2t", [4, 8, 128, 1024], BF16)  # W2 [j-block, j, p, 8x128]

    outT_d = nc.dram_tensor("outT", [CT, 128, 2 * TB], F32, kind="ExternalOutput").ap()

    payl = [
        nc.dram_tensor(f"payl{b}", [NCORES * 128, TB + 2], BF16).ap() for b in range(B)
    ]
    gath = [
        nc.dram_tensor(f"gath{b}", [NCORES * 128, TB + 2], BF16).ap() for b in range(B)
    ]

    with tile.TileContext(nc) as tc:
        with tc.tile_pool(name="cst", bufs=1) as cst:
            b1t = cst.tile([128, NJ], F32)
            nc.sync.dma_start(b1t[:], b1_d[:])
            b2t = cst.tile([128, CT], F32)
            nc.sync.dma_start(b2t[:], b2_d[:])
            g2t = cst.tile([128, CT], F32)
            nc.sync.dma_start(g2t[:], g2_d[:])
            be2t = cst.tile([128, CT], F32)
            nc.sync.dma_start(be2t[:], be2_d[:])

            # W1 tiles live for the whole kernel; each DMA WAR-waits on a tiny
            # memset issued mid-batch-0 so the transfers start only after the
            # x-input stream has drained (no HBM contention on the ramp).
            w1sb = {}
            for jb in range(4):
                for i in range(CT):
                    w_ = cst.tile([128, 1024], BF16, name=f"w1_{i}_{jb}",
                                  tag=f"w1_{i}_{jb}")
                    w1sb[(i, jb)] = w_

            # ---------------- PHASE A ----------------
            with (
                tc.tile_pool(name="ac", bufs=1) as ac,
                tc.tile_pool(name="xtp", bufs=16) as xtp,
                tc.tile_pool(name="wsp", bufs=1) as wsp,
                tc.tile_pool(name="qkp", bufs=1) as qkp,
                tc.tile_pool(name="vp", bufs=1) as vp,
                tc.tile_pool(name="weip", bufs=8) as weip,
                tc.tile_pool(name="xhp", bufs=1) as xhp,
                tc.tile_pool(name="sp", bufs=2) as sp,
                tc.tile_pool(name="psmm", bufs=2, space="PSUM") as psmm,
                tc.tile_pool(name="psat", bufs=1, space="PSUM") as psat,
                tc.tile_pool(name="psax", bufs=2, space="PSUM") as psax,
            ):
                g1t = ac.tile([128, CT], F32)
                nc.sync.dma_start(g1t[:], g1_d[:])
                be1t = ac.tile([128, CT], F32)
                nc.sync.dma_start(be1t[:], be1_d[:])
                tri = ac.tile([128, 128], BF16)
                nc.sync.dma_start(tri[:], tm_d[:])
                idn = ac.tile([128, 128], BF16)
                nc.sync.dma_start(idn[:], id_d[:])
                onr = ac.tile([1, 64], BF16)
                nc.sync.dma_start(onr[:], onr_d[:])
                onc = ac.tile([128, 1], BF16)
                nc.sync.dma_start(onc[:], onc_d[:])
                wraw = {}
                for pname, pd in (("qq", wqq_d), ("kk", wkk_d), ("vv", wvv_d)):
                    w = ac.tile([128, C], BF16, name=f"wr_{pname}", tag=f"wr_{pname}")
                    nc.sync.dma_start(w[:], pd[:])
                    wraw[pname] = w

                xh = {}  # x1 per (b, head-half), [64, T] bf16
                for b in range(B):
                    xts = []
                    for ct in range(CT):
                        t_ = xtp.tile([128, T], BF16, name="xt", tag="xt")
                        nc.sync.dma_start(t_[:], xb_d[b, ct])
                        xts.append(t_)
                    for hl in range(2):
                        t_ = xhp.tile(
                            [64, T], BF16, name=f"xh_{b}_{hl}", tag=f"xh_{b}_{hl}"
                        )
                        nc.sync.dma_start(t_[:], xow_d[b, hl])
                        xh[(b, hl)] = t_

                    # -- LN1 stats -> s1cat/bbcat [128, 8] columns
                    s1cat = sp.tile([128, CT], F32, tag="s1cat")
                    bbcat = sp.tile([128, CT], F32, tag="bbcat")
                    bbb = sp.tile([128, CT], BF16, tag="bbb")
                    ws = {
                        pname: wsp.tile(
                            [128, C], BF16, name=f"ws_{pname}", tag=f"ws_{pname}"
                        )
                        for pname in ("qq", "kk", "vv")
                    }
                    for ct in range(CT):
                        st6 = sp.tile([128, 4, 6], F32, tag="st6")
                        for i in range(4):
                            nc.vector.bn_stats(
                                st6[:, i, :], xts[ct][:, i * TS : (i + 1) * TS]
                            )
                        mv = sp.tile([128, 2], F32, tag="mv")
                        nc.vector.bn_aggr(mv[:], st6[:])
                        va = sp.tile([128, 1], F32, tag="va")
                        nc.vector.tensor_scalar(
                            out=va[:], in0=mv[:, 1:2], scalar1=VCORR, scalar2=EPS,
                            op0=ALU.mult, op1=ALU.add,
                        )
                        sq = sp.tile([128, 1], F32, tag="sq")
                        nc.scalar.activation(sq[:], va[:], AF.Sqrt)
                        rs = sp.tile([128, 1], F32, tag="rs")
                        nc.vector.reciprocal_approx_fast(out=rs[:], in_=sq[:])
                        s1 = s1cat[:, ct : ct + 1]
                        nc.vector.tensor_mul(s1, g1t[:, ct : ct + 1], rs[:])
                        bb = bbcat[:, ct : ct + 1]
                        nc.vector.tensor_mul(bb, mv[:, 0:1], s1)
                        nc.vector.tensor_sub(bb, be1t[:, ct : ct + 1], bb)
                        # fold LN1 into the weights per-ct (DVE is in-order, so
                        # interleaving keeps QKV pipelined with the x DMA+stats
                        # instead of gating on all 8 channel tiles at once)
                        nc.vector.tensor_copy(
                            bbb[:, ct : ct + 1], bbcat[:, ct : ct + 1]
                        )
                        for pname in ("qq", "kk", "vv"):
                            nc.vector.tensor_mul(
                                ws[pname][:, 128 * ct : 128 * (ct + 1)],
                                wraw[pname][:, 128 * ct : 128 * (ct + 1)],
                                s1cat[:, ct : ct + 1].broadcast_to((128, 128)),
                            )

                    bcol = {}
                    for pname in ("qq", "kk", "vv"):
                        bp_ = psax.tile([128, 1], F32, name="bps", tag="aux")
                        for ct in range(CT):
                            nc.tensor.matmul(
                                bp_[:], wraw[pname][:, 128 * ct : 128 * (ct + 1)],
                                bbb[:, ct : ct + 1],
                                start=(ct == 0), stop=(ct == CT - 1),
                            )
                        bc = sp.tile([128, 1], F32, tag=f"bc_{pname}")
                        nc.vector.tensor_copy(bc[:], bp_[:])
                        bcol[pname] = bc

                    # -- QKV streams
                    packT = {}
                    for pname in ("qq", "kk", "vv"):
                        o_ = qkp.tile(
                            [128, T], BF16, name=f"{pname}T", tag=f"{pname}T"
                        )
                        for ts in range(NTS):
                            ps = psmm.tile([128, TS], F32, name="mm", tag="mm")
                            for ct in range(CT):
                                nc.tensor.matmul(
                                    ps[:], ws[pname][:, 128 * ct : 128 * (ct + 1)],
                                    xts[ct][:, ts * TS : (ts + 1) * TS],
                                    start=(ct == 0), stop=(ct == CT - 1),
                                )
                            nc.vector.tensor_add(
                                o_[:, ts * TS : (ts + 1) * TS], ps[:],
                                bcol[pname][:].broadcast_to((128, TS)),
                            )
                        packT[pname] = o_

                    if b == 0:
                        # gate W1 loads behind batch-0's QKV: the memsets sit
                        # here in the DVE stream, so the gpsimd DMA transfers
                        # (WAR on them) begin only after the x ramp drained
                        for jb in range(4):
                            for i in range(CT):
                                nc.vector.memset(w1sb[(i, jb)][:, 0:1], 0.0)
                                nc.gpsimd.dma_start(
                                    w1sb[(i, jb)][:], w1_d[i, jb]
                                )

                    # -- V transpose to token-major packed tiles [128, 2, 65]
                    v2 = {}
                    for st in range(NST):
                        pt_ = psax.tile([128, 128], BF16, name="vT", tag="aux")
                        nc.tensor.transpose(
                            pt_[:], packT["vv"][:, st * 128 : (st + 1) * 128], idn[:]
                        )
                        v_ = vp.tile(
                            [128, 2, 65], BF16, name=f"v2_{st}", tag=f"v2_{st}"
                        )
                        nc.vector.tensor_copy(
                            v_[:, :, 0:64],
                            pt_[:].rearrange("p (h d) -> p h d", d=64),
                        )
                        nc.vector.tensor_copy(
                            v_[:, :, 64:65], onc[:, :, None].broadcast_to((128, 2, 1))
                        )
                        v2[st] = v_

                    # -- attention, head-halves interleaved, t-slices in pairs;
                    #    attnV runs one st behind scores (software pipeline) so
                    #    the PE never waits on the Exp of the current st.
                    st6b = {}
                    for hl in range(2):
                        st6b[hl] = sp.tile(
                            [64, 4, 6], F32, name=f"st6b{hl}", tag=f"st6b{hl}"
                        )

                    def flush(pend, b=b, xh=xh, st6b=st6b):
                        for (hl, ts), (we, w0, pst) in pend.items():
                            nc.tensor.matmul(
                                aps[(hl, ts)][:, w0:TS], v2[pst][:, hl, :], we[:],
                                start=(pst == 0), stop=(pst == 4 * ts + 3),
                            )
                        for (hl, ts), (we, w0, pst) in pend.items():
                            if 4 * ts + 3 != pst:
                                continue
                            ap_ = aps[(hl, ts)]
                            den = sp.tile([1, TS], F32, tag="den")
                            nc.vector.tensor_copy(den[:], ap_[64:65, :])
                            rec = sp.tile([1, TS], F32, tag="rec")
                            nc.vector.reciprocal_approx_fast(
                                out=rec[:], in_=den[:]
                            )
                            recb = sp.tile([1, TS], BF16, tag="recb")
                            nc.vector.tensor_copy(recb[:], rec[:])
                            rbp = psax.tile([64, TS], F32, name="rb", tag="aux")
                            nc.tensor.matmul(
                                rbp[:], onr[:], recb[:], start=True, stop=True
                            )
                            rb = sp.tile([64, TS], F32, tag="rb")
                            nc.vector.tensor_copy(rb[:], rbp[:])
                            tmp = sp.tile([64, TS], F32, tag="tmp")
                            nc.vector.tensor_mul(tmp[:], ap_[0:64, :], rb[:])
                            xs = xh[(b, hl)][:, ts * TS : (ts + 1) * TS]
                            nc.vector.tensor_add(xs, xs, tmp[:])
                            # LN2 stats for this slice, then ship its chunks
                            nc.vector.bn_stats(st6b[hl][:, ts, :], xs)
                            for half in range(2):
                                j = 2 * ts + half
                                nc.sync.dma_start(
                                    payl[b][
                                        128 * j + 64 * hl :
                                        128 * j + 64 * hl + 64, 0:TB
                                    ],
                                    xh[(b, hl)][
                                        :, ts * TS + half * TB :
                                        ts * TS + (half + 1) * TB
                                    ],
                                )

                    for tpair in range(2):
                        ts_list = [2 * tpair, 2 * tpair + 1]
                        aps = {}
                        for hl in range(2):
                            for ts in ts_list:
                                tg = 2 * hl + (ts - 2 * tpair)
                                aps[(hl, ts)] = psat.tile(
                                    [65, TS], F32, name=f"at{tg}", tag=f"at{tg}"
                                )
                        pending = {}
                        for st in range(8 * tpair + 8):
                            ts0 = st // 4
                            off = 128 * (st % 4)
                            weis = {}
                            for hl in range(2):
                                lo = 64 * hl
                                for ts in ts_list:
                                    if ts < ts0:
                                        continue
                                    w0 = off if ts == ts0 else 0
                                    wid = TS - w0
                                    wp = psmm.tile(
                                        [128, wid], F32, name="mm", tag="mm"
                                    )
                                    nc.tensor.matmul(
                                        wp[:],
                                        packT["kk"][
                                            lo : lo + 64, st * 128 : (st + 1) * 128
                                        ],
                                        packT["qq"][
                                            lo : lo + 64, ts * TS + w0 : (ts + 1) * TS
                                        ],
                                        start=True, stop=True,
                                    )
                                    we = weip.tile(
                                        [128, wid], BF16, name="wei", tag="wei"
                                    )
                                    nc.scalar.activation(
                                        we[:], wp[:], AF.Exp, scale=SCALE
                                    )
                                    if ts == ts0:
                                        nc.vector.tensor_mul(
                                            we[:, 0:128], we[:, 0:128], tri[:]
                                        )
                                    weis[(hl, ts)] = (we, w0, st)
                            flush(pending)
                            pending = weis
                        flush(pending)

                    # -- LN2 aggregate -> payload stat columns
                    for hl in range(2):
                        mv = sp.tile([64, 2], F32, tag="mv2")
                        st6 = st6b[hl]
                        nc.vector.bn_aggr(mv[:], st6[:])
                        sb2 = sp.tile([64, 2], BF16, tag="sb2")
                        nc.vector.tensor_copy(sb2[:, 0:1], mv[:, 0:1])
                        va = sp.tile([64, 1], F32, tag="va2")
                        nc.vector.tensor_scalar(
                            out=va[:], in0=mv[:, 1:2], scalar1=VCORR, scalar2=EPS,
                            op0=ALU.mult, op1=ALU.add,
                        )
                        nc.vector.tensor_copy(sb2[:, 1:2], va[:])
                        for j in range(NCORES):
                            nc.sync.dma_start(
                                payl[b][
                                    128 * j + 64 * hl : 128 * j + 64 * hl + 64,
                                    TB : TB + 2,
                                ],
                                sb2[:],
                            )

                    # -- per-batch collective, triggered as soon as batch done
                    nc.gpsimd.collective_compute(
                        "AllToAll",
                        ALU.bypass,
                        ins=[payl[b][:]],
                        outs=[gath[b][:]],
                        replica_groups=[list(range(NCORES))],
                    )

            # ---------------- PHASE B: MLP on own 2x256 tokens ----------------
            with (
                tc.tile_pool(name="wp", bufs=1) as wp,
                tc.tile_pool(name="bp", bufs=1) as bp,
                tc.tile_pool(name="h1p", bufs=1) as h1p,
                tc.tile_pool(name="y2p", bufs=1) as y2p,
                tc.tile_pool(name="psB", bufs=4, space="PSUM") as psB,
            ):
                # W2 tiles (SBUF space reuses phase A's via stack-scoped pool);
                # DMAs are emitted after the y2 section below so their memset
                # WAR-gates delay the transfers until the second collective has
                # finished (no network/DMA contention with it).
                w2sb = {}
                for jb in range(4):
                    for j in range(8):
                        w_ = wp.tile([128, 1024], BF16, name=f"w2_{jb}_{j}",
                                     tag=f"w2_{jb}_{j}")
                        w2sb[(jb, j)] = w_

                # gather -> x1g tiles [128, 2, TB+2] (slot per batch)
                x1g = []
                for i in range(CT):
                    t_ = bp.tile([128, 2, TB + 2], BF16, name=f"x1g{i}", tag=f"x1g{i}")
                    for b in range(B):
                        nc.sync.dma_start(
                            t_[:, b, :], gath[b][128 * i : 128 * (i + 1), :]
                        )
                    x1g.append(t_)

                # receiver-side LN2: rstd from shipped (mean, var), then y2
                y2 = []
                for i in range(CT):
                    mva = bp.tile([128, 2, 2], F32, name=f"mva{i}", tag=f"mva{i}")
                    nc.vector.tensor_copy(mva[:], x1g[i][:, :, TB : TB + 2])
                    sq = bp.tile([128, 2], F32, name=f"sqB{i}", tag=f"sqB{i}")
                    nc.scalar.activation(sq[:], mva[:, :, 1], AF.Sqrt)
                    rs = bp.tile([128, 2], F32, name=f"rsB{i}", tag=f"rsB{i}")
                    nc.vector.reciprocal_approx_fast(out=rs[:], in_=sq[:])
                    sb = bp.tile([128, 2, 2], F32, name=f"sbB{i}", tag=f"sbB{i}")
                    s2 = sb[:, :, 0]
                    nc.vector.tensor_mul(
                        s2, g2t[:, i : i + 1].broadcast_to((128, 2)), rs[:]
                    )
                    b2_ = sb[:, :, 1]
                    nc.vector.tensor_mul(b2_, mva[:, :, 0], s2)
                    nc.vector.tensor_sub(
                        b2_, be2t[:, i : i + 1].broadcast_to((128, 2)), b2_
                    )
                    t_ = y2p.tile([128, 2, TB], BF16, name=f"y2{i}", tag=f"y2{i}")
                    for b in range(B):
                        nc.scalar.activation(
                            t_[:, b, :], x1g[i][:, b, 0:TB], AF.Identity,
                            scale=sb[:, b, 0:1], bias=sb[:, b, 1:2],
                        )
                    y2.append(t_)

                # W2 loads, gated behind the second collective: the memsets sit
                # after the gath-dependent y2 chain in the DVE stream
                for jb in range(4):
                    for j in range(8):
                        nc.vector.memset(w2sb[(jb, j)][:, 0:1], 0.0)
                        nc.gpsimd.dma_start(w2sb[(jb, j)][:], w2_d[jb, j])

                # h1 = relu(y2 @ W1 + b1): all 32 tiles resident (bf16)
                h1 = []
                for jb in range(4):
                    for j in range(8):
                        o = 128 * j
                        ps = psB.tile([128, TS], F32, name="hm", tag="hm", bufs=2)
                        for i in range(CT):
                            nc.tensor.matmul(
                                ps[:].rearrange("p (b t) -> p b t", b=2),
                                w1sb[(i, jb)][:, o : o + 128],
                                y2[i][:],
                                start=(i == 0), stop=(i == CT - 1),
                            )
                        h_ = h1p.tile(
                            [128, TS], BF16, name=f"h1_{jb}_{j}", tag=f"h1_{jb}_{j}"
                        )
                        nc.scalar.activation(
                            h_[:], ps[:], AF.Relu, bias=b1t[:, 8 * jb + j : 8 * jb + j + 1]
                        )
                        h1.append(h_)

                # out = h1 @ W2 + b2 + x1 -- finish each k-tile, write it out
                for k in range(CT):
                    ps = psB.tile([128, TS], F32, name="om", tag="om", bufs=2)
                    for jb in range(4):
                        for jx in range(8):
                            nc.tensor.matmul(
                                ps[:],
                                w2sb[(jb, jx)][:, 128 * k : 128 * (k + 1)],
                                h1[8 * jb + jx][:],
                                start=(jb == 0 and jx == 0),
                                stop=(jb == 3 and jx == 7),
                            )
                    mo = bp.tile([128, TS], F32, name="mo", tag="mo")
                    nc.scalar.activation(
                        mo[:], ps[:], AF.Identity, bias=b2t[:, k : k + 1]
                    )
                    oo = bp.tile([128, TS], F32, name="oo", tag="oo")
                    nc.vector.tensor_add(
                        oo[:].rearrange("p (b t) -> p b t", b=2),
                        mo[:].rearrange("p (b t) -> p b t", b=2),
                        x1g[k][:, :, 0:TB],
                    )
                    nc.sync.dma_start(outT_d[k], oo[:])

    nc.compile()
    return nc


def _prep(inputs):
    import ml_dtypes

    BFNP = ml_dtypes.bfloat16

    x = np.asarray(inputs["x"], np.float32)
    Wq = np.asarray(inputs["Wq"], np.float32)
    Wk = np.asarray(inputs["Wk"], np.float32)
    Wv = np.asarray(inputs["Wv"], np.float32)
    W1 = np.asarray(inputs["W1"], np.float32)
    W2 = np.asarray(inputs["W2"], np.float32)
    b1 = np.asarray(inputs["b1"], np.float32)
    b2 = np.asarray(inputs["b2"], np.float32)
    g1 = np.asarray(inputs["g1"], np.float32)
    be1 = np.asarray(inputs["be1"], np.float32)
    g2 = np.asarray(inputs["g2"], np.float32)
    be2 = np.asarray(inputs["be2"], np.float32)

    # channel-major bf16 x: [B, CT, 128, T]
    xb = np.ascontiguousarray(
        x.reshape(B, T, CT, 128).transpose(0, 2, 3, 1)
    ).astype(BFNP)

    t_idx = np.arange(128)[None, :]
    p_idx = np.arange(128)[:, None]
    trimask = (t_idx >= p_idx).astype(BFNP)

    w1t = (
        np.ascontiguousarray(W1.reshape(CT, 128, 4, 1024).transpose(0, 2, 1, 3))
        .astype(BFNP)
    )
    w2t = np.ascontiguousarray(W2.reshape(4, 8, 128, 1024)).astype(BFNP)

    def packc(Wa, Wb):
        # [128, 8*128] where col block ct = rows 128ct..128ct+128 of [Wa|Wb]
        p = np.concatenate([Wa, Wb], axis=1)  # [1024, 128]
        return np.ascontiguousarray(
            p.reshape(CT, 128, 128).transpose(1, 0, 2).reshape(128, C)
        ).astype(BFNP)

    shared = {
        "xb": xb,
        "g1c": np.ascontiguousarray(g1.reshape(CT, 128).T),
        "be1c": np.ascontiguousarray(be1.reshape(CT, 128).T),
        "g2c": np.ascontiguousarray(g2.reshape(CT, 128).T),
        "be2c": np.ascontiguousarray(be2.reshape(CT, 128).T),
        "b1c": np.ascontiguousarray(b1.reshape(NJ, 128).T),
        "b2c": np.ascontiguousarray(b2.reshape(CT, 128).T),
        "trimask": trimask,
        "identr": np.eye(128, dtype=BFNP),
        "onesrow": np.ones((1, 64), BFNP),
        "onescol": np.ones((128, 1), BFNP),
        "w1t": w1t,
        "w2t": w2t,
    }
    in_maps = []
    for c in range(NCORES):
        h0, h1_ = 2 * c, 2 * c + 1
        m = dict(shared)
        m["wqq"] = packc(Wq[h0], Wq[h1_])
        m["wkk"] = packc(Wk[h0], Wk[h1_])
        m["wvv"] = packc(Wv[h0], Wv[h1_])
        m["xow"] = np.ascontiguousarray(xb[:, c].reshape(B, 2, 64, T))
        in_maps.append(m)
    return in_maps


def kernel(**inputs) -> np.ndarray:
    if "nc" not in _CACHE:
        _CACHE["nc"] = build()
    nc = _CACHE["nc"]
    in_maps = _prep(inputs)
    res = run_bass_kernel_spmd(nc, in_maps, core_ids=list(range(NCORES)))
    out = np.empty((B, T, C), np.float32)
    for c in range(NCORES):
        oT = res.results[c]["outT"]  # [8, 128, 512]: cols 0:256 b0, 256:512 b1
        for b in range(B):
            blk = oT[:, :, b * TB : (b + 1) * TB]  # [CT, 128, TB]
            out[b, TB * c : TB * (c + 1), :] = (
                blk.transpose(2, 0, 1).reshape(TB, C)
            )
    return out


# revision 28
# speedup vs baseline: 1.0119x; 1.0119x over previous
"""Fused transformer block (LN-over-sequence + causal MHA + LN + MLP, residuals)
for Trainium2, distributed over 8 NeuronCores.

v2 design (all-bf16 matmul paths; measured rel-err ~5e-3 vs 2e-2 gate):
  Phase A (attention): tensor-parallel over heads -- core c owns heads
    (2c, 2c+1) = channels [128c, 128c+128). Channel-major layouts throughout
    so the sequence-dim LayerNorms reduce along the free axis.
    LN1 is folded into the QKV weights (bf16); x is uploaded once in bf16.
    Softmax without max-subtraction (logits bounded); denominator from an
    appended ones-column in V (attention-psum row 64); reciprocal via the
    fast DVE approx; causality via partial-width diagonal slices + one
    triangular mask multiply.
  Re-shard: per-batch bf16 AllToAlls (payload = x1 + LN2 stats columns).
    Batch-0's collective is triggered as soon as batch-0's attention ends and
    overlaps batch-1 compute; token ownership is interleaved (core c owns
    tokens [256c, 256c+256) of BOTH batches) so the SPMD program reads both
    gather buffers symmetrically.
  Phase B (MLP): token-parallel 1024->4096->1024 on the core's 512 tokens
    (256 per batch, column-packed). LN2 scale/bias are computed receiver-side
    from shipped (mean, var) so the scalar engine's Exp table never swaps
    during attention. W1/W2 are DMA'd at kernel start on the GpSimd queue so
    they never block the input/payload stream.
"""

import numpy as np

import concourse.bass as bass
import concourse.mybir as mybir
import concourse.tile as tile
from concourse import bacc
from concourse.bass_utils import run_bass_kernel_spmd

B, T, C, H, D = 2, 2048, 1024, 16, 64
NCORES = 8
CT = C // 128  # 8 channel tiles
TS = 512  # token slice width (attention loop)
NTS = T // TS  # 4
NST = T // 128  # 16 s-tiles
TB = T // NCORES  # 256 tokens owned per (core, batch)
HID = 4 * C
NJ = HID // 128  # 32 hidden tiles
EPS = 1e-5
VCORR = T / (T - 1.0)  # unbiased-variance correction (torch.var ddof=1)
SCALE = C**-0.5  # attention logit scale = 1/32

F32 = mybir.dt.float32
F32R = mybir.dt.float32r
BF16 = mybir.dt.bfloat16
AF = mybir.ActivationFunctionType
ALU = mybir.AluOpType

_CACHE: dict = {}


def build():
    nc = bacc.Bacc(None, target_bir_lowering=False, debug=False, num_devices=NCORES)

    def din(name, shape, dt=F32):
        return nc.dram_tensor(name, shape, dt, kind="ExternalInput").ap()

    xb_d = din("xb", [B, CT, 128, T], BF16)  # x channel-major, bf16 (shared)
    xow_d = din("xow", [B, 2, 64, T], BF16)  # core's own channels
    # packs laid out [128, 8*128]: col block ct holds rows 128ct..128ct+128
    wqq_d = din("wqq", [128, C], BF16)
    wkk_d = din("wkk", [128, C], BF16)
    wvv_d = din("wvv", [128, C], BF16)
    g1_d = din("g1c", [128, CT], F32)
    be1_d = din("be1c", [128, CT], F32)
    g2_d = din("g2c", [128, CT], F32)
    be2_d = din("be2c", [128, CT], F32)
    b1_d = din("b1c", [128, NJ], F32)
    b2_d = din("b2c", [128, CT], F32)
    tm_d = din("trimask", [128, 128], BF16)  # tri: 1 if t_local >= s_local
    id_d = din("identr", [128, 128], BF16)
    onr_d = din("onesrow", [1, 64], BF16)
    onc_d = din("onescol", [128, 1], BF16)
    w1_d = din("w1t", [CT, 4, 128, 1024], BF16)  # W1 [c-tile, jb, p, 8x128]
    w2_d = din("w2t", [4, 8, 128, 1024], BF16)  # W2 [j-block, j, p, 8x128]

    outT_d = nc.dram_tensor("outT", [CT, 128, 2 * TB], F32, kind="ExternalOutput").ap()

    payl = [
        nc.dram_tensor(f"payl{b}", [NCORES * 128, TB + 2], BF16).ap() for b in range(B)
    ]
    gath = [
        nc.dram_tensor(f"gath{b}", [NCORES * 128, TB + 2], BF16).ap() for b in range(B)
    ]

    with tile.TileContext(nc) as tc:
        with tc.tile_pool(name="cst", bufs=1) as cst:
            b1t = cst.tile([128, NJ], F32)
            nc.sync.dma_start(b1t[:], b1_d[:])
            b2t = cst.tile([128, CT], F32)
            nc.sync.dma_start(b2t[:], b2_d[:])
            g2t = cst.tile([128, CT], F32)
            nc.sync.dma_start(g2t[:], g2_d[:])
            be2t = cst.tile([128, CT], F32)
            nc.sync.dma_start(be2t[:], be2_d[:])

            # W1 tiles live for the whole kernel; each DMA WAR-waits on a tiny
            # memset issued mid-batch-0 so the transfers start only after the
            # x-input stream has drained (no HBM contention on the ramp).
            w1sb = {}
            for jb in range(4):
                for i in range(CT):
                    w_ = cst.tile([128, 1024], BF16, name=f"w1_{i}_{jb}",
                                  tag=f"w1_{i}_{jb}")
                    w1sb[(i, jb)] = w_

            # ---------------- PHASE A ----------------
            with (
                tc.tile_pool(name="ac", bufs=1) as ac,
                tc.tile_pool(name="xtp", bufs=16) as xtp,
                tc.tile_pool(name="wsp", bufs=1) as wsp,
                tc.tile_pool(name="qkp", bufs=1) as qkp,
                tc.tile_pool(name="vp", bufs=1) as vp,
                tc.tile_pool(name="weip", bufs=8) as weip,
                tc.tile_pool(name="xhp", bufs=1) as xhp,
                tc.tile_pool(name="sp", bufs=2) as sp,
                tc.tile_pool(name="psmm", bufs=2, space="PSUM") as psmm,
                tc.tile_pool(name="psat", bufs=1, space="PSUM") as psat,
                tc.tile_pool(name="psax", bufs=2, space="PSUM") as psax,
            ):
                g1t = ac.tile([128, CT], F32)
                nc.sync.dma_start(g1t[:], g1_d[:])
                be1t = ac.tile([128, CT], F32)
                nc.sync.dma_start(be1t[:], be1_d[:])
                tri = ac.tile([128, 128], BF16)
                nc.sync.dma_start(tri[:], tm_d[:])
                idn = ac.tile([128, 128], BF16)
                nc.sync.dma_start(idn[:], id_d[:])
                onr = ac.tile([1, 64], BF16)
                nc.sync.dma_start(onr[:], onr_d[:])
                onc = ac.tile([128, 1], BF16)
                nc.sync.dma_start(onc[:], onc_d[:])
                wraw = {}
                for pname, pd in (("qq", wqq_d), ("kk", wkk_d), ("vv", wvv_d)):
                    w = ac.tile([128, C], BF16, name=f"wr_{pname}", tag=f"wr_{pname}")
                    nc.sync.dma_start(w[:], pd[:])
                    wraw[pname] = w

                xh = {}  # x1 per (b, head-half), [64, T] bf16
                for b in range(B):
                    xts = []
                    for ct in range(CT):
                        t_ = xtp.tile([128, T], BF16, name="xt", tag="xt")
                        nc.sync.dma_start(t_[:], xb_d[b, ct])
                        xts.append(t_)
                    for hl in range(2):
                        t_ = xhp.tile(
                            [64, T], BF16, name=f"xh_{b}_{hl}", tag=f"xh_{b}_{hl}"
                        )
                        nc.sync.dma_start(t_[:], xow_d[b, hl])
                        xh[(b, hl)] = t_

                    # -- LN1 stats -> s1cat/bbcat [128, 8] columns
                    s1cat = sp.tile([128, CT], F32, tag="s1cat")
                    bbcat = sp.tile([128, CT], F32, tag="bbcat")
                    bbb = sp.tile([128, CT], BF16, tag="bbb")
                    ws = {
                        pname: wsp.tile(
                            [128, C], BF16, name=f"ws_{pname}", tag=f"ws_{pname}"
                        )
                        for pname in ("qq", "kk", "vv")
                    }
                    for ct in range(CT):
                        st6 = sp.tile([128, 4, 6], F32, tag="st6")
                        for i in range(4):
                            nc.vector.bn_stats(
                                st6[:, i, :], xts[ct][:, i * TS : (i + 1) * TS]
                            )
                        mv = sp.tile([128, 2], F32, tag="mv")
                        nc.vector.bn_aggr(mv[:], st6[:])
                        va = sp.tile([128, 1], F32, tag="va")
                        nc.vector.tensor_scalar(
                            out=va[:], in0=mv[:, 1:2], scalar1=VCORR, scalar2=EPS,
                            op0=ALU.mult, op1=ALU.add,
                        )
                        sq = sp.tile([128, 1], F32, tag="sq")
                        nc.scalar.activation(sq[:], va[:], AF.Sqrt)
                        rs = sp.tile([128, 1], F32, tag="rs")
                        nc.vector.reciprocal_approx_fast(out=rs[:], in_=sq[:])
                        s1 = s1cat[:, ct : ct + 1]
                        nc.vector.tensor_mul(s1, g1t[:, ct : ct + 1], rs[:])
                        bb = bbcat[:, ct : ct + 1]
                        nc.vector.tensor_mul(bb, mv[:, 0:1], s1)
                        nc.vector.tensor_sub(bb, be1t[:, ct : ct + 1], bb)
                        # fold LN1 into the weights per-ct (DVE is in-order, so
                        # interleaving keeps QKV pipelined with the x DMA+stats
                        # instead of gating on all 8 channel tiles at once)
                        nc.vector.tensor_copy(
                            bbb[:, ct : ct + 1], bbcat[:, ct : ct + 1]
                        )
                        for pname in ("qq", "kk", "vv"):
                            nc.vector.tensor_mul(
                                ws[pname][:, 128 * ct : 128 * (ct + 1)],
                                wraw[pname][:, 128 * ct : 128 * (ct + 1)],
                                s1cat[:, ct : ct + 1].broadcast_to((128, 128)),
                            )

                    bcol = {}
                    for pname in ("qq", "kk", "vv"):
                        bp_ = psax.tile([128, 1], F32, name="bps", tag="aux")
                        for ct in range(CT):
                            nc.tensor.matmul(
                                bp_[:], wraw[pname][:, 128 * ct : 128 * (ct + 1)],
                                bbb[:, ct : ct + 1],
                                start=(ct == 0), stop=(ct == CT - 1),
                            )
                        bc = sp.tile([128, 1], F32, tag=f"bc_{pname}")
                        nc.vector.tensor_copy(bc[:], bp_[:])
                        bcol[pname] = bc

                    # -- QKV streams
                    packT = {}
                    for pname in ("qq", "kk", "vv"):
                        o_ = qkp.tile(
                            [128, T], BF16, name=f"{pname}T", tag=f"{pname}T"
                        )
                        for ts in range(NTS):
                            ps = psmm.tile([128, TS], F32, name="mm", tag="mm")
                            for ct in range(CT):
                                nc.tensor.matmul(
                                    ps[:], ws[pname][:, 128 * ct : 128 * (ct + 1)],
                                    xts[ct][:, ts * TS : (ts + 1) * TS],
                                    start=(ct == 0), stop=(ct == CT - 1),
                                )
                            nc.vector.tensor_add(
                                o_[:, ts * TS : (ts + 1) * TS], ps[:],
                                bcol[pname][:].broadcast_to((128, TS)),
                            )
                        packT[pname] = o_

                    if b == 0:
                        # gate W1 loads behind batch-0's QKV: the memsets sit
                        # here in the DVE stream, so the gpsimd DMA transfers
                        # (WAR on them) begin only after the x ramp drained
                        for jb in range(4):
                            for i in range(CT):
                                nc.vector.memset(w1sb[(i, jb)][:, 0:1], 0.0)
                                nc.gpsimd.dma_start(
                                    w1sb[(i, jb)][:], w1_d[i, jb]
                                )

                    # -- V transpose to token-major packed tiles [128, 2, 65]
                    v2 = {}
                    for st in range(NST):
                        pt_ = psax.tile([128, 128], BF16, name="vT", tag="aux")
                        nc.tensor.transpose(
                            pt_[:], packT["vv"][:, st * 128 : (st + 1) * 128], idn[:]
                        )
                        v_ = vp.tile(
                            [128, 2, 65], BF16, name=f"v2_{st}", tag=f"v2_{st}"
                        )
                        nc.vector.tensor_copy(
                            v_[:, :, 0:64],
                            pt_[:].rearrange("p (h d) -> p h d", d=64),
                        )
                        nc.vector.tensor_copy(
                            v_[:, :, 64:65], onc[:, :, None].broadcast_to((128, 2, 1))
                        )
                        v2[st] = v_

                    # -- attention, head-halves interleaved, t-slices in pairs;
                    #    attnV runs one st behind scores (software pipeline) so
                    #    the PE never waits on the Exp of the current st.
                    st6b = {}
                    for hl in range(2):
                        st6b[hl] = sp.tile(
                            [64, 4, 6], F32, name=f"st6b{hl}", tag=f"st6b{hl}"
                        )

                    def flush(pend, b=b, xh=xh, st6b=st6b):
                        for (hl, ts), (we, w0, pst) in pend.items():
                            nc.tensor.matmul(
                                aps[(hl, ts)][:, w0:TS], v2[pst][:, hl, :], we[:],
                                start=(pst == 0), stop=(pst == 4 * ts + 3),
                            )
                        for (hl, ts), (we, w0, pst) in pend.items():
                            if 4 * ts + 3 != pst:
                                continue
                            ap_ = aps[(hl, ts)]
                            den = sp.tile([1, TS], F32, tag="den")
                            nc.vector.tensor_copy(den[:], ap_[64:65, :])
                            rec = sp.tile([1, TS], F32, tag="rec")
                            nc.vector.reciprocal_approx_fast(
                                out=rec[:], in_=den[:]
                            )
                            recb = sp.tile([1, TS], BF16, tag="recb")
                            nc.vector.tensor_copy(recb[:], rec[:])
                            rbp = psax.tile([64, TS], F32, name="rb", tag="aux")
                            nc.tensor.matmul(
                                rbp[:], onr[:], recb[:], start=True, stop=True
                            )
                            rb = sp.tile([64, TS], F32, tag="rb")
                            nc.vector.tensor_copy(rb[:], rbp[:])
                            tmp = sp.tile([64, TS], F32, tag="tmp")
                            nc.vector.tensor_mul(tmp[:], ap_[0:64, :], rb[:])
                            xs = xh[(b, hl)][:, ts * TS : (ts + 1) * TS]
                            nc.vector.tensor_add(xs, xs, tmp[:])
                            # LN2 stats for this slice, then ship its chunks
                            nc.vector.bn_stats(st6b[hl][:, ts, :], xs)
                            for half in range(2):
                                j = 2 * ts + half
                                nc.sync.dma_start(
                                    payl[b][
                                        128 * j + 64 * hl :
                                        128 * j + 64 * hl + 64, 0:TB
                                    ],
                                    xh[(b, hl)][
                                        :, ts * TS + half * TB :
                                        ts * TS + (half + 1) * TB
                                    ],
                                )

                    for tpair in range(2):
                        ts_list = [2 * tpair, 2 * tpair + 1]
                        aps = {}
                        for hl in range(2):
                            for ts in ts_list:
                                tg = 2 * hl + (ts - 2 * tpair)
                                aps[(hl, ts)] = psat.tile(
                                    [65, TS], F32, name=f"at{tg}", tag=f"at{tg}"
                                )
                        pending = {}
                        for st in range(8 * tpair + 8):
                            ts0 = st // 4
                            off = 128 * (st % 4)
                            weis = {}
                            for hl in range(2):
                                lo = 64 * hl
                                for ts in ts_list:
                                    if ts < ts0:
                                        continue
                                    w0 = off if ts == ts0 else 0
                                    wid = TS - w0
                                    wp = psmm.tile(
                                        [128, wid], F32, name="mm", tag="mm"
                                    )
                                    nc.tensor.matmul(
                                        wp[:],
                                        packT["kk"][
                                            lo : lo + 64, st * 128 : (st + 1) * 128
                                        ],
                                        packT["qq"][
                                            lo : lo + 64, ts * TS + w0 : (ts + 1) * TS
                                        ],
                                        start=True, stop=True,
                                    )
                                    we = weip.tile(
                                        [128, wid], BF16, name="wei", tag="wei"
                                    )
                                    nc.scalar.activation(
                                        we[:], wp[:], AF.Exp, scale=SCALE
                                    )
                                    if ts == ts0:
                                        nc.vector.tensor_mul(
                                            we[:, 0:128], we[:, 0:128], tri[:]
                                        )
                                    weis[(hl, ts)] = (we, w0, st)
                            flush(pending)
                            pending = weis
                        flush(pending)

                    # -- LN2 aggregate -> payload stat columns
                    for hl in range(2):
                        mv = sp.tile([64, 2], F32, tag="mv2")
                        st6 = st6b[hl]
                        nc.vector.bn_aggr(mv[:], st6[:])
                        sb2 = sp.tile([64, 2], BF16, tag="sb2")
                        nc.vector.tensor_copy(sb2[:, 0:1], mv[:, 0:1])
                        va = sp.tile([64, 1], F32, tag="va2")
                        nc.vector.tensor_scalar(
                            out=va[:], in0=mv[:, 1:2], scalar1=VCORR, scalar2=EPS,
                            op0=ALU.mult, op1=ALU.add,
                        )
                        nc.vector.tensor_copy(sb2[:, 1:2], va[:])
                        for j in range(NCORES):
                            nc.sync.dma_start(
                                payl[b][
                                    128 * j + 64 * hl : 128 * j + 64 * hl + 64,
                                    TB : TB + 2,
                                ],
                                sb2[:],
                            )

                    # -- per-batch collective, triggered as soon as batch done
                    nc.gpsimd.collective_compute(
                        "AllToAll",
                        ALU.bypass,
                        ins=[payl[b][:]],
                        outs=[gath[b][:]],
                        replica_groups=[list(range(NCORES))],
                    )

            # ---------------- PHASE B: MLP on own 2x256 tokens ----------------
            with (
                tc.tile_pool(name="wp", bufs=1) as wp,
                tc.tile_pool(name="bp", bufs=1) as bp,
                tc.tile_pool(name="h1p", bufs=1) as h1p,
                tc.tile_pool(name="y2p", bufs=1) as y2p,
                tc.tile_pool(name="psB", bufs=4, space="PSUM") as psB,
            ):
                # W2 tiles (SBUF space reuses phase A's via stack-scoped pool);
                # DMAs are emitted after the y2 section below so their memset
                # WAR-gates delay the transfers until the second collective has
                # finished (no network/DMA contention with it).
                w2sb = {}
                for jb in range(4):
                    for j in range(8):
                        w_ = wp.tile([128, 1024], BF16, name=f"w2_{jb}_{j}",
                                     tag=f"w2_{jb}_{j}")
                        nc.gpsimd.dma_start(w_[:], w2_d[jb, j])
                        w2sb[(jb, j)] = w_

                # gather -> x1g tiles [128, 2, TB+2] (slot per batch)
                x1g = []
                for i in range(CT):
                    t_ = bp.tile([128, 2, TB + 2], BF16, name=f"x1g{i}", tag=f"x1g{i}")
                    for b in range(B):
                        nc.sync.dma_start(
                            t_[:, b, :], gath[b][128 * i : 128 * (i + 1), :]
                        )
                    x1g.append(t_)

                # receiver-side LN2: rstd from shipped (mean, var), then y2
                y2 = []
                for i in range(CT):
                    mva = bp.tile([128, 2, 2], F32, name=f"mva{i}", tag=f"mva{i}")
                    nc.vector.tensor_copy(mva[:], x1g[i][:, :, TB : TB + 2])
                    sq = bp.tile([128, 2], F32, name=f"sqB{i}", tag=f"sqB{i}")
                    nc.scalar.activation(sq[:], mva[:, :, 1], AF.Sqrt)
                    rs = bp.tile([128, 2], F32, name=f"rsB{i}", tag=f"rsB{i}")
                    nc.vector.reciprocal_approx_fast(out=rs[:], in_=sq[:])
                    sb = bp.tile([128, 2, 2], F32, name=f"sbB{i}", tag=f"sbB{i}")
                    s2 = sb[:, :, 0]
                    nc.vector.tensor_mul(
                        s2, g2t[:, i : i + 1].broadcast_to((128, 2)), rs[:]
                    )
                    b2_ = sb[:, :, 1]
                    nc.vector.tensor_mul(b2_, mva[:, :, 0], s2)
                    nc.vector.tensor_sub(
                        b2_, be2t[:, i : i + 1].broadcast_to((128, 2)), b2_
                    )
                    t_ = y2p.tile([128, 2, TB], BF16, name=f"y2{i}", tag=f"y2{i}")
                    for b in range(B):
                        nc.scalar.activation(
                            t_[:, b, :], x1g[i][:, b, 0:TB], AF.Identity,
                            scale=sb[:, b, 0:1], bias=sb[:, b, 1:2],
                        )
                    y2.append(t_)

                # h1 = relu(y2 @ W1 + b1): all 32 tiles resident (bf16)
                h1 = []
                for jb in range(4):
                    for j in range(8):
                        o = 128 * j
                        ps = psB.tile([128, TS], F32, name="hm", tag="hm", bufs=2)
                        for i in range(CT):
                            nc.tensor.matmul(
                                ps[:].rearrange("p (b t) -> p b t", b=2),
                                w1sb[(i, jb)][:, o : o + 128],
                                y2[i][:],
                                start=(i == 0), stop=(i == CT - 1),
                            )
                        h_ = h1p.tile(
                            [128, TS], BF16, name=f"h1_{jb}_{j}", tag=f"h1_{jb}_{j}"
                        )
                        nc.scalar.activation(
                            h_[:], ps[:], AF.Relu, bias=b1t[:, 8 * jb + j : 8 * jb + j + 1]
                        )
                        h1.append(h_)

                # out = h1 @ W2 + b2 + x1 -- finish each k-tile, write it out
                for k in range(CT):
                    ps = psB.tile([128, TS], F32, name="om", tag="om", bufs=2)
                    for jb in range(4):
                        for jx in range(8):
                            nc.tensor.matmul(
                                ps[:],
                                w2sb[(jb, jx)][:, 128 * k : 128 * (k + 1)],
                                h1[8 * jb + jx][:],
                                start=(jb == 0 and jx == 0),
                                stop=(jb == 3 and jx == 7),
                            )
                    mo = bp.tile([128, TS], F32, name="mo", tag="mo")
                    nc.scalar.activation(
                        mo[:], ps[:], AF.Identity, bias=b2t[:, k : k + 1]
                    )
                    oo = bp.tile([128, TS], F32, name="oo", tag="oo")
                    nc.vector.tensor_add(
                        oo[:].rearrange("p (b t) -> p b t", b=2),
                        mo[:].rearrange("p (b t) -> p b t", b=2),
                        x1g[k][:, :, 0:TB],
                    )
                    nc.sync.dma_start(outT_d[k], oo[:])

    nc.compile()
    return nc


def _prep(inputs):
    import ml_dtypes

    BFNP = ml_dtypes.bfloat16

    x = np.asarray(inputs["x"], np.float32)
    Wq = np.asarray(inputs["Wq"], np.float32)
    Wk = np.asarray(inputs["Wk"], np.float32)
    Wv = np.asarray(inputs["Wv"], np.float32)
    W1 = np.asarray(inputs["W1"], np.float32)
    W2 = np.asarray(inputs["W2"], np.float32)
    b1 = np.asarray(inputs["b1"], np.float32)
    b2 = np.asarray(inputs["b2"], np.float32)
    g1 = np.asarray(inputs["g1"], np.float32)
    be1 = np.asarray(inputs["be1"], np.float32)
    g2 = np.asarray(inputs["g2"], np.float32)
    be2 = np.asarray(inputs["be2"], np.float32)

    # channel-major bf16 x: [B, CT, 128, T]
    xb = np.ascontiguousarray(
        x.reshape(B, T, CT, 128).transpose(0, 2, 3, 1)
    ).astype(BFNP)

    t_idx = np.arange(128)[None, :]
    p_idx = np.arange(128)[:, None]
    trimask = (t_idx >= p_idx).astype(BFNP)

    w1t = (
        np.ascontiguousarray(W1.reshape(CT, 128, 4, 1024).transpose(0, 2, 1, 3))
        .astype(BFNP)
    )
    w2t = np.ascontiguousarray(W2.reshape(4, 8, 128, 1024)).astype(BFNP)

    def packc(Wa, Wb):
        # [128, 8*128] where col block ct = rows 128ct..128ct+128 of [Wa|Wb]
        p = np.concatenate([Wa, Wb], axis=1)  # [1024, 128]
        return np.ascontiguousarray(
            p.reshape(CT, 128, 128).transpose(1, 0, 2).reshape(128, C)
        ).astype(BFNP)

    shared = {
        "xb": xb,
        "g1c": np.ascontiguousarray(g1.reshape(CT, 128).T),
        "be1c": np.ascontiguousarray(be1.reshape(CT, 128).T),
        "g2c": np.ascontiguousarray(g2.reshape(CT, 128).T),
        "be2c": np.ascontiguousarray(be2.reshape(CT, 128).T),
        "b1c": np.ascontiguousarray(b1.reshape(NJ, 128).T),
        "b2c": np.ascontiguousarray(b2.reshape(CT, 128).T),
        "trimask": trimask,
        "identr": np.eye(128, dtype=BFNP),
        "onesrow": np.ones((1, 64), BFNP),
        "onescol": np.ones((128, 1), BFNP),
        "w1t": w1t,
        "w2t": w2t,
    }
    in_maps = []
    for c in range(NCORES):
        h0, h1_ = 2 * c, 2 * c + 1
        m = dict(shared)
        m["wqq"] = packc(Wq[h0], Wq[h1_])
        m["wkk"] = packc(Wk[h0], Wk[h1_])
        m["wvv"] = packc(Wv[h0], Wv[h1_])
        m["xow"] = np.ascontiguousarray(xb[:, c].reshape(B, 2, 64, T))
        in_maps.append(m)
    return in_maps


def kernel(**inputs) -> np.ndarray:
    if "nc" not in _CACHE:
        _CACHE["nc"] = build()
    nc = _CACHE["nc"]
    in_maps = _prep(inputs)
    res = run_bass_kernel_spmd(nc, in_maps, core_ids=list(range(NCORES)))
    out = np.empty((B, T, C), np.float32)
    for c in range(NCORES):
        oT = res.results[c]["outT"]  # [8, 128, 512]: cols 0:256 b0, 256:512 b1
        for b in range(B):
            blk = oT[:, :, b * TB : (b + 1) * TB]  # [CT, 128, TB]
            out[b, TB * c : TB * (c + 1), :] = (
                blk.transpose(2, 0, 1).reshape(TB, C)
            )
    return out
